# revision 17
# baseline (speedup 1.0000x reference)
"""Trainium2 Bass kernel for a single-layer transformer block (attention + FFN + 2x LayerNorm).

Shapes (hardcoded): q,k,v [4,4096,128] fp32; w1 [128,512]; w2 [512,128]; out [4,4096,128].

Sharding: 8 cores; core c handles batch c//2, q-rows half c%2 (2048 rows each).
k/v for the batch are replicated on both cores of the pair. Pure data-parallel SPMD,
no collectives.

Host-side marshalling (inside kernel(), before the device kernel runs): q and k are
pre-transposed to [d, rows] layout and cast to bf16, v/w2 are pre-tiled into the
[128-partition, tile, 128] layout the PE consumes, so the device kernel does ZERO
on-chip transposes or dtype casts. The output is produced transposed ([d, rows] bf16)
and the host transposes/casts it back.

Per-core device algorithm (activations TRANSPOSED: [feature/kpos on partitions, rows free]):
  - slot over 2 kpos tiles: scores_T[kpos, rows] = kT_blk.T @ qT into a 2-bank psum
    tile (double-buffered: exp(n) overlaps scores(n+1); a single-buffered 4-bank tile
    was tried and serializes scores->exp->scores, costing ~1.2us/slot of PE idle),
    then ONE exp over the 1024 free elements (amortizes the ~550ns fixed ACT cost).
  - P = exp(scores / sqrt(d))    (max-subtraction unneeded: logits ~N(0,1); the
    softmax denominator cancels in LayerNorm scale-invariance)
  - attn_T[d, rows] += v_blk.T.T @ P_blk   (PE accumulation, bf16)
  - LN over d (=partitions): mean/meansq via ones-matmul with M=128 so the stats land
    REPLICATED across all partitions (no broadcast matmul); rstd = rsqrt(var+eps) via
    the bf16 quake bit-trick + one f32 Newton step, entirely on DVE (keeps ACT free
    for exp, which is the bottleneck engine).
  - FFN: h1T = w1.T @ xT (+b1, relu on DVE), ffnT = w2_blk.T @ h1T accumulated (PE).
  - residual + LN2, DMA the transposed bf16 result straight out.

q blocks are 512,512,512,256,256 columns: the small final block shortens the kernel
tail (its post-LN/FFN chain has no attention stream left to hide behind), and its
post phase is further split into two 128-column chains that pipeline against each
other.  Post-attention ops of block i spread across the attention slots of block i+1.
"""

import sys

sys.path.insert(0, "/opt/trn_rl_repo")

from collections import deque
from contextlib import ExitStack

import ml_dtypes
import numpy as np

import concourse.bass as bass  # noqa: F401
from concourse import bacc
import concourse.tile as tile
import concourse.mybir as mybir
from concourse.bass_utils import run_bass_kernel_spmd

B, S, D, F = 4, 4096, 128, 512
N_CORES = 8
HALF = S // 2          # q rows per core
NKT = S // 128         # 32 kpos tiles
FBLK = F // 128        # 4 FFN chunks
EPS = 1e-5
INV_SQRT_D = float(1.0 / np.sqrt(D))
# (start, end, kpos-tiles-per-slot): narrow late blocks shorten the kernel tail;
# widening their slots (4 kpos tiles x 256 cols = same 1024-elem exp, same 2 psum
# banks) keeps the per-element ACT exp cost identical to the 512-col blocks.
BLOCKS = [
    (0, 512, 2),
    (512, 1024, 2),
    (1024, 1536, 2),
    (1536, 1792, 4),
    (1792, 2048, 4),
]

f32 = mybir.dt.float32
bf16 = mybir.dt.bfloat16
u16 = mybir.dt.uint16
AF = mybir.ActivationFunctionType
ALU = mybir.AluOpType

# quake rsqrt magic for bf16 (top 16 bits of the f32 magic 0x5f3759df)
QMAGIC = 0x5F37


def _emit(nc, tc, ctx):
    qT = nc.dram_tensor("qT", [D, HALF], bf16, kind="ExternalInput")
    kT = nc.dram_tensor("kT", [D, S], bf16, kind="ExternalInput")
    vt = nc.dram_tensor("vt", [128, NKT, D], bf16, kind="ExternalInput")
    w1 = nc.dram_tensor("w1", [D, F], bf16, kind="ExternalInput")
    w2c = nc.dram_tensor("w2c", [128, FBLK, D], bf16, kind="ExternalInput")
    b1c = nc.dram_tensor("b1c", [128, FBLK], f32, kind="ExternalInput")
    b2 = nc.dram_tensor("b2", [D], f32, kind="ExternalInput")
    g1 = nc.dram_tensor("g1", [D], f32, kind="ExternalInput")
    be1 = nc.dram_tensor("be1", [D], f32, kind="ExternalInput")
    g2 = nc.dram_tensor("g2", [D], f32, kind="ExternalInput")
    be2 = nc.dram_tensor("be2", [D], f32, kind="ExternalInput")
    outT = nc.dram_tensor("outT", [D, HALF], bf16, kind="ExternalOutput")

    # ---------------- pools ----------------
    persist = ctx.enter_context(tc.tile_pool(name="persist", bufs=1))
    p_pool = ctx.enter_context(tc.tile_pool(name="p", bufs=3))
    xz_pool = ctx.enter_context(tc.tile_pool(name="xz", bufs=4))
    x_pool = ctx.enter_context(tc.tile_pool(name="x", bufs=4))
    h_pool = ctx.enter_context(tc.tile_pool(name="h", bufs=4))
    st_pool = ctx.enter_context(tc.tile_pool(name="st", bufs=6))
    y_pool = ctx.enter_context(tc.tile_pool(name="y", bufs=3))

    # PSUM: score 2x2 banks (double-buffered so exp(n) overlaps scores(n+1))
    # + acc 2x1 + misc 2x1 = 8 banks exactly.
    score_ps = ctx.enter_context(tc.tile_pool(name="score_ps", bufs=2, space="PSUM"))
    acc_ps = ctx.enter_context(tc.tile_pool(name="acc_ps", bufs=2, space="PSUM"))
    misc_ps = ctx.enter_context(tc.tile_pool(name="misc_ps", bufs=2, space="PSUM"))

    # ---------------- big activations, startup-ordered DMAs ----------------
    kT_sb = persist.tile([128, S], bf16, tag="kT")
    qT_sb = persist.tile([128, HALF], bf16, tag="qT")
    v_sb = persist.tile([128, NKT, D], bf16, tag="v")

    # The first super-slot needs kT tiles 0-3 and qT block 0; feed the queue in
    # consumption order with fine chunks so the PE starts after ~200KB.
    nc.sync.dma_start(out=kT_sb[:, 0:512], in_=kT[:, 0:512])
    nc.sync.dma_start(out=qT_sb[:, 0:512], in_=qT[:, 0:512])
    nc.sync.dma_start(out=kT_sb[:, 512:1024], in_=kT[:, 512:1024])
    nc.sync.dma_start(out=v_sb[:, 0:8, :], in_=vt[:, 0:8, :])
    nc.sync.dma_start(out=kT_sb[:, 1024:2048], in_=kT[:, 1024:2048])
    nc.sync.dma_start(out=v_sb[:, 8:16, :], in_=vt[:, 8:16, :])
    nc.sync.dma_start(out=kT_sb[:, 2048:3072], in_=kT[:, 2048:3072])
    nc.sync.dma_start(out=v_sb[:, 16:24, :], in_=vt[:, 16:24, :])
    nc.sync.dma_start(out=kT_sb[:, 3072:S], in_=kT[:, 3072:S])
    nc.sync.dma_start(out=v_sb[:, 24:NKT, :], in_=vt[:, 24:NKT, :])
    nc.sync.dma_start(out=qT_sb[:, 512:HALF], in_=qT[:, 512:HALF])

    w1_sb = persist.tile([128, F], bf16, tag="w1")
    nc.sync.dma_start(out=w1_sb, in_=w1[:, :])
    w2_sb = persist.tile([128, FBLK, D], bf16, tag="w2")
    nc.sync.dma_start(out=w2_sb, in_=w2c[:, :, :])
    b1_sb = persist.tile([128, FBLK], f32, tag="b1")
    nc.sync.dma_start(out=b1_sb, in_=b1c[:, :])
    b2_t = persist.tile([128, 1], f32, tag="b2")
    nc.sync.dma_start(out=b2_t, in_=b2.ap().unsqueeze(1))
    g1_t = persist.tile([128, 1], f32, tag="g1")
    nc.sync.dma_start(out=g1_t, in_=g1.ap().unsqueeze(1))
    be1_t = persist.tile([128, 1], f32, tag="be1")
    nc.sync.dma_start(out=be1_t, in_=be1.ap().unsqueeze(1))
    g2_t = persist.tile([128, 1], f32, tag="g2")
    nc.sync.dma_start(out=g2_t, in_=g2.ap().unsqueeze(1))
    be2_t = persist.tile([128, 1], f32, tag="be2")
    nc.sync.dma_start(out=be2_t, in_=be2.ap().unsqueeze(1))

    # ---------------- constants ----------------
    # ones/D with M=128 so stat matmuls produce the mean replicated on every
    # partition -> no separate partition-broadcast step.
    ones_mm = persist.tile([128, 128], bf16, tag="ones_mm")
    nc.gpsimd.memset(ones_mm, 1.0 / D)

    # ---------------- post-attention phase as spreadable op list ----------------
    def make_post_ops(rows0, xz, c0, c1, act_rstd=False):
        """Closures for LN1 + FFN + residual + LN2 + store of columns [c0:c1) of
        the block whose attention output (and its square) sit in xz [128,2,*] bf16.

        act_rstd=True computes rstd via ACT Ln/Exp instead of the DVE quake
        sequence — used ONLY for the final block's chains, where ACT is idle
        (no exps left) and the DVE queue is the latency bottleneck."""
        N = c1 - c0
        cols = slice(c0, c1)
        st = {}
        ops = []

        def ln_ops(src0, src1, dst_key, g_t, be_t, key):
            """LayerNorm over partitions; src0/src1 are lambdas returning [128,N]
            APs (value and value^2); normalized result lands in st[dst_key].
            rstd comes from the bf16 quake rsqrt + one f32 Newton step on DVE."""

            def s1():
                st[key + "mu"] = misc_ps.tile([128, N], f32, tag="misc", name="ps_mu")
                nc.tensor.matmul(st[key + "mu"], ones_mm, src0())

            def s2():
                st[key + "ms"] = misc_ps.tile([128, N], f32, tag="misc", name="ps_ms")
                nc.tensor.matmul(st[key + "ms"], ones_mm, src1())

            def s3():
                t = st_pool.tile([128, 2, N], bf16, tag="st", name="st")
                st[key + "st"] = t
                nc.vector.tensor_copy(t[:, 0, :], st[key + "mu"])

            def s4():
                t = st[key + "st"]
                # var+eps = (ms + eps) - mu^2, into the bf16 stat tile
                nc.vector.tensor_tensor(t[:, 1, :], t[:, 0, :], t[:, 0, :], ALU.mult)
                nc.vector.scalar_tensor_tensor(
                    t[:, 1, :], st[key + "ms"], EPS, t[:, 1, :], ALU.add, ALU.subtract
                )

            def s5():
                # quake: y0 = bitcast(QMAGIC - (bits(var) >> 1)) in bf16-space
                t = st[key + "st"]
                vbits = t[:, 1, :].bitcast(u16)
                q0 = st_pool.tile([128, N], bf16, tag="q0", name="q0")
                q0b = q0.bitcast(u16)
                # QMAGIC - x == (x ^ 0xFFFF) - (0xFFFF - QMAGIC); intermediates
                # stay in [0, 0xFFFF] whether uint16 math wraps or saturates.
                # (bitwise and arith ALU ops can't be fused in one instruction)
                nc.vector.tensor_scalar(q0b, vbits, 1, None, ALU.logical_shift_right)
                nc.vector.tensor_scalar(q0b, q0b, 0xFFFF, None, ALU.bitwise_xor)
                nc.vector.tensor_scalar(q0b, q0b, 0xFFFF - QMAGIC, None, ALU.subtract)
                st[key + "q0"] = q0

            def s5b():
                # one Newton step in f32: rstd = y0*(1.5 - 0.5*var*y0^2)
                t = st[key + "st"]
                q0 = st[key + "q0"]
                tn = st_pool.tile([128, N], f32, tag="tn", name="tn")
                nc.vector.tensor_tensor(tn, t[:, 1, :], q0, ALU.mult)
                nc.vector.tensor_tensor(tn, tn, q0, ALU.mult)
                nc.vector.tensor_scalar(tn, tn, -0.5, 1.5, ALU.mult, ALU.add)
                nc.vector.tensor_tensor(tn, q0, tn, ALU.mult)
                st[key + "rstd"] = tn

            def s5_act():
                # rstd = exp(-0.5 * ln(var + eps)) on the otherwise-idle ACT
                t = st[key + "st"]
                nc.scalar.activation(t[:, 1, :], t[:, 1, :], AF.Ln)
                nc.scalar.activation(t[:, 1, :], t[:, 1, :], AF.Exp, scale=-0.5)
                st[key + "rstd"] = t[:, 1, :]

            def s6():
                t = st[key + "st"]
                dst = st[dst_key]
                nc.vector.tensor_tensor(dst, src0(), t[:, 0, :], ALU.subtract)
                nc.vector.tensor_tensor(dst, dst, st[key + "rstd"], ALU.mult)
                nc.vector.tensor_scalar(dst, dst, g_t, be_t, ALU.mult, ALU.add)

            if act_rstd:
                return [s1, s2, s3, s4, s5_act, s6]
            return [s1, s2, s3, s4, s5, s5b, s6]

        def a0():
            st["x1"] = x_pool.tile([128, N], bf16, tag="x", name="x1")

        ops.append(a0)
        ops.extend(
            ln_ops(lambda: xz[:, 0, cols], lambda: xz[:, 1, cols], "x1", g1_t, be1_t, "l1")
        )

        def f0():
            st["ffn"] = acc_ps.tile([128, N], f32, tag="acc", name="ps_ffn")

        ops.append(f0)
        for fb in range(FBLK):
            def fchunk(fb=fb):
                ps_h = misc_ps.tile([128, N], f32, tag="misc", name="ps_h")
                nc.tensor.matmul(ps_h, w1_sb[:, fb * 128 : (fb + 1) * 128], st["x1"])
                h = h_pool.tile([128, N], bf16, tag="h", name="h")
                if act_rstd:
                    # tail: ACT is idle, DVE is the latency bottleneck
                    nc.scalar.activation(h, ps_h, AF.Relu, bias=b1_sb[:, fb : fb + 1])
                else:
                    # relu(x + b1): fused add+max on DVE keeps ACT free for exp
                    nc.vector.tensor_scalar(h, ps_h, b1_sb[:, fb : fb + 1], 0.0, ALU.add, ALU.max)
                nc.tensor.matmul(
                    st["ffn"],
                    w2_sb[:, fb, :],
                    h,
                    start=(fb == 0),
                    stop=(fb == FBLK - 1),
                    skip_group_check=True,
                )

            ops.append(fchunk)

        def r0():
            z = xz_pool.tile([128, 2, N], bf16, tag="xz", name="z")
            st["z"] = z
            nc.vector.scalar_tensor_tensor(z[:, 0, :], st["ffn"], b2_t, st["x1"], ALU.add, ALU.add)
            if act_rstd:
                nc.scalar.activation(z[:, 1, :], z[:, 0, :], AF.Square)
            else:
                nc.vector.tensor_tensor(z[:, 1, :], z[:, 0, :], z[:, 0, :], ALU.mult)
            st["y2"] = y_pool.tile([128, N], bf16, tag="y", name="y2")

        ops.append(r0)
        ops.extend(
            ln_ops(lambda: st["z"][:, 0, :], lambda: st["z"][:, 1, :], "y2", g2_t, be2_t, "l2")
        )

        def out_dma():
            nc.sync.dma_start(out=outT[:, rows0 + c0 : rows0 + c1], in_=st["y2"])

        ops.append(out_dma)
        return ops

    # ---------------- software-pipelined main loop ----------------
    pending = deque()  # post ops of the previous block

    def pop(n=1):
        for _ in range(n):
            if pending:
                pending.popleft()()

    for bi, (r0, r1, ks) in enumerate(BLOCKS):
        N = r1 - r0
        nsup = NKT // ks
        ps_attn = acc_ps.tile([128, N], f32, tag="acc")
        prev_p = None
        prev_base = 0
        for sp in range(nsup):
            ps_s = score_ps.tile([128, ks, N], f32, tag="score")
            for hh in range(ks):
                jk = ks * sp + hh
                nc.tensor.matmul(
                    ps_s[:, hh, :], kT_sb[:, jk * 128 : (jk + 1) * 128], qT_sb[:, r0:r1]
                )
                if hh == 1:
                    pop()
            p_sb = p_pool.tile([128, ks, N], bf16, tag="p")
            nc.scalar.activation(p_sb, ps_s, AF.Exp, scale=INV_SQRT_D)
            # One-slot skew: accumulate the PREVIOUS slot's P@v so the PE never
            # waits on this slot's exp.
            if prev_p is not None:
                for hh in range(ks):
                    jk = prev_base + hh
                    nc.tensor.matmul(
                        ps_attn,
                        v_sb[:, jk, :],
                        prev_p[:, hh, :],
                        start=(jk == 0),
                        stop=False,
                        skip_group_check=True,
                    )
                    if hh == 1:
                        pop()
            prev_p = p_sb
            prev_base = ks * sp
            pop(2 if bi == len(BLOCKS) - 1 else 1)
        for hh in range(ks):  # drain the skewed last slot
            jk = prev_base + hh
            nc.tensor.matmul(
                ps_attn,
                v_sb[:, jk, :],
                prev_p[:, hh, :],
                start=False,
                stop=(hh == ks - 1),
                skip_group_check=True,
            )
        # Eagerly spill the attention accumulator (and its square for the LN1
        # stats) so its psum bank frees for the next block. Remaining post ops
        # carry over into the next block's slots instead of clumping.
        xz = xz_pool.tile([128, 2, N], bf16, tag="xz", name="xz")
        if bi == len(BLOCKS) - 1:
            # final block: exps are done, spill via the idle ACT engine
            nc.scalar.activation(xz[:, 0, :], ps_attn, AF.Copy)
            nc.scalar.activation(xz[:, 1, :], ps_attn, AF.Square)
        else:
            nc.vector.tensor_copy(xz[:, 0, :], ps_attn)
            nc.vector.tensor_tensor(xz[:, 1, :], xz[:, 0, :], xz[:, 0, :], ALU.mult)
        if bi < len(BLOCKS) - 1:
            pending.extend(make_post_ops(r0, xz, 0, N))
        else:
            # split the final block's post phase into two half-width chains so the
            # kernel tail pipelines instead of one long dependency chain; rstd on
            # the now-idle ACT engine to unload the DVE queue
            opsA = make_post_ops(r0, xz, 0, N // 2, act_rstd=True)
            opsB = make_post_ops(r0, xz, N // 2, N, act_rstd=True)
            for a, b in zip(opsA, opsB):
                pending.append(a)
                pending.append(b)
    while pending:
        pending.popleft()()


def _patched_act_tables(module_arch):
    """Collapse the ACT table choice to the one set containing exp (+relu/copy
    fillers) so the kernel never swaps table sets (~2.7us per swap). Positions are
    preserved because act_func_set_id indexes the original act_info.json order."""
    from concourse.hw_specs import get_activation_tables

    tables = get_activation_tables(module_arch)
    keep = "natural_log_exp_and_others"
    if keep in tables:
        return {
            name: (funcs if name == keep else set())
            for name, funcs in tables.items()
        }
    return tables


def build():
    nc = bacc.Bacc("TRN2", target_bir_lowering=False, debug=False, num_devices=N_CORES)
    with tile.TileContext(nc) as tc:
        with ExitStack() as ctx:
            _emit(nc, tc, ctx)
    import concourse.bacc as bacc_mod

    orig = bacc_mod.get_activation_tables
    bacc_mod.get_activation_tables = _patched_act_tables
    try:
        nc.compile()
    finally:
        bacc_mod.get_activation_tables = orig
    return nc


_CACHE = {}


def _get_nc():
    if "nc" not in _CACHE:
        _CACHE["nc"] = build()
    return _CACHE["nc"]


def run(inputs, trace=False, trace_kwargs=None):
    """Run on 8 cores; returns (full_output, BassKernelResults)."""
    nc = _get_nc()
    bf = ml_dtypes.bfloat16
    q = np.asarray(inputs["q"], dtype=np.float32)
    k = np.asarray(inputs["k"], dtype=np.float32)
    v = np.asarray(inputs["v"], dtype=np.float32)
    w1_b = np.asarray(inputs["w1"], dtype=np.float32).astype(bf)
    w2c = (
        np.asarray(inputs["w2"], dtype=np.float32)
        .reshape(FBLK, 128, D)
        .transpose(1, 0, 2)
        .astype(bf)
    )
    b1c = np.ascontiguousarray(
        np.asarray(inputs["b1"], dtype=np.float32).reshape(FBLK, 128).T
    )
    scal = {
        n: np.ascontiguousarray(np.asarray(inputs[n], dtype=np.float32))
        for n in ("b2", "g1", "be1", "g2", "be2")
    }
    in_maps = []
    for c in range(N_CORES):
        b, h = divmod(c, 2)
        m = {"w1": w1_b, "w2c": w2c, "b1c": b1c, **scal}
        m["qT"] = np.ascontiguousarray(q[b, h * HALF : (h + 1) * HALF, :].T).astype(bf)
        m["kT"] = np.ascontiguousarray(k[b].T).astype(bf)
        m["vt"] = v[b].reshape(NKT, 128, D).transpose(1, 0, 2).astype(bf)
        in_maps.append(m)
    res = run_bass_kernel_spmd(
        nc, in_maps, list(range(N_CORES)), trace=trace, **(trace_kwargs or {})
    )
    full = np.empty((B, S, D), dtype=np.float32)
    for c in range(N_CORES):
        b, h = divmod(c, 2)
        full[b, h * HALF : (h + 1) * HALF, :] = (
            res.results[c]["outT"].astype(np.float32).T
        )
    return full, res


def kernel(**inputs):
    full, _ = run(inputs, trace=False)
    return full


# revision 18
# speedup vs baseline: 1.0016x; 1.0016x over previous
"""Trainium2 Bass kernel for a single-layer transformer block (attention + FFN + 2x LayerNorm).

Shapes (hardcoded): q,k,v [4,4096,128] fp32; w1 [128,512]; w2 [512,128]; out [4,4096,128].

Sharding: 8 cores; core c handles batch c//2, q-rows half c%2 (2048 rows each).
k/v for the batch are replicated on both cores of the pair. Pure data-parallel SPMD,
no collectives.

Host-side marshalling (inside kernel(), before the device kernel runs): q and k are
pre-transposed to [d, rows] layout and cast to bf16, v/w2 are pre-tiled into the
[128-partition, tile, 128] layout the PE consumes, so the device kernel does ZERO
on-chip transposes or dtype casts. The output is produced transposed ([d, rows] bf16)
and the host transposes/casts it back.

Per-core device algorithm (activations TRANSPOSED: [feature/kpos on partitions, rows free]):
  - slot over 2 kpos tiles: scores_T[kpos, rows] = kT_blk.T @ qT into a 2-bank psum
    tile (double-buffered: exp(n) overlaps scores(n+1); a single-buffered 4-bank tile
    was tried and serializes scores->exp->scores, costing ~1.2us/slot of PE idle),
    then ONE exp over the 1024 free elements (amortizes the ~550ns fixed ACT cost).
  - P = exp(scores / sqrt(d))    (max-subtraction unneeded: logits ~N(0,1); the
    softmax denominator cancels in LayerNorm scale-invariance)
  - attn_T[d, rows] += v_blk.T.T @ P_blk   (PE accumulation, bf16)
  - LN over d (=partitions): mean/meansq via ones-matmul with M=128 so the stats land
    REPLICATED across all partitions (no broadcast matmul); rstd = rsqrt(var+eps) via
    the bf16 quake bit-trick + one f32 Newton step, entirely on DVE (keeps ACT free
    for exp, which is the bottleneck engine).
  - FFN: h1T = w1.T @ xT (+b1, relu on DVE), ffnT = w2_blk.T @ h1T accumulated (PE).
  - residual + LN2, DMA the transposed bf16 result straight out.

q blocks are 512,512,512,256,256 columns: the small final block shortens the kernel
tail (its post-LN/FFN chain has no attention stream left to hide behind), and its
post phase is further split into two 128-column chains that pipeline against each
other.  Post-attention ops of block i spread across the attention slots of block i+1.
"""

import sys

sys.path.insert(0, "/opt/trn_rl_repo")

from collections import deque
from contextlib import ExitStack

import ml_dtypes
import numpy as np

import concourse.bass as bass  # noqa: F401
from concourse import bacc
import concourse.tile as tile
import concourse.mybir as mybir
from concourse.bass_utils import run_bass_kernel_spmd

B, S, D, F = 4, 4096, 128, 512
N_CORES = 8
HALF = S // 2          # q rows per core
NKT = S // 128         # 32 kpos tiles
FBLK = F // 128        # 4 FFN chunks
EPS = 1e-5
INV_SQRT_D = float(1.0 / np.sqrt(D))
# (start, end, kpos-tiles-per-slot): narrow late blocks shorten the kernel tail;
# widening their slots (4 kpos tiles x 256 cols = same 1024-elem exp, same 2 psum
# banks) keeps the per-element ACT exp cost identical to the 512-col blocks.
BLOCKS = [
    (0, 512, 2),
    (512, 1024, 2),
    (1024, 1536, 2),
    (1536, 1792, 4),
    (1792, 2048, 4),
]

f32 = mybir.dt.float32
bf16 = mybir.dt.bfloat16
u16 = mybir.dt.uint16
AF = mybir.ActivationFunctionType
ALU = mybir.AluOpType

# quake rsqrt magic for bf16 (top 16 bits of the f32 magic 0x5f3759df)
QMAGIC = 0x5F37


def _emit(nc, tc, ctx):
    qT = nc.dram_tensor("qT", [D, HALF], bf16, kind="ExternalInput")
    kT = nc.dram_tensor("kT", [D, S], bf16, kind="ExternalInput")
    vt = nc.dram_tensor("vt", [128, NKT, D], bf16, kind="ExternalInput")
    w1 = nc.dram_tensor("w1", [D, F], bf16, kind="ExternalInput")
    w2c = nc.dram_tensor("w2c", [128, FBLK, D], bf16, kind="ExternalInput")
    b1c = nc.dram_tensor("b1c", [128, FBLK], f32, kind="ExternalInput")
    b2 = nc.dram_tensor("b2", [D], f32, kind="ExternalInput")
    g1 = nc.dram_tensor("g1", [D], f32, kind="ExternalInput")
    be1 = nc.dram_tensor("be1", [D], f32, kind="ExternalInput")
    g2 = nc.dram_tensor("g2", [D], f32, kind="ExternalInput")
    be2 = nc.dram_tensor("be2", [D], f32, kind="ExternalInput")
    outT = nc.dram_tensor("outT", [D, HALF], bf16, kind="ExternalOutput")

    # ---------------- pools ----------------
    persist = ctx.enter_context(tc.tile_pool(name="persist", bufs=1))
    p_pool = ctx.enter_context(tc.tile_pool(name="p", bufs=3))
    xz_pool = ctx.enter_context(tc.tile_pool(name="xz", bufs=4))
    x_pool = ctx.enter_context(tc.tile_pool(name="x", bufs=4))
    h_pool = ctx.enter_context(tc.tile_pool(name="h", bufs=4))
    st_pool = ctx.enter_context(tc.tile_pool(name="st", bufs=6))
    y_pool = ctx.enter_context(tc.tile_pool(name="y", bufs=3))

    # PSUM: score 2x2 banks (double-buffered so exp(n) overlaps scores(n+1))
    # + acc 2x1 + misc 2x1 = 8 banks exactly.
    score_ps = ctx.enter_context(tc.tile_pool(name="score_ps", bufs=2, space="PSUM"))
    acc_ps = ctx.enter_context(tc.tile_pool(name="acc_ps", bufs=2, space="PSUM"))
    misc_ps = ctx.enter_context(tc.tile_pool(name="misc_ps", bufs=2, space="PSUM"))

    # ---------------- big activations, startup-ordered DMAs ----------------
    kT_sb = persist.tile([128, S], bf16, tag="kT")
    qT_sb = persist.tile([128, HALF], bf16, tag="qT")
    v_sb = persist.tile([128, NKT, D], bf16, tag="v")

    # The first super-slot needs kT tiles 0-3 and qT block 0; feed the queue in
    # consumption order with fine chunks so the PE starts after ~200KB.
    nc.sync.dma_start(out=kT_sb[:, 0:512], in_=kT[:, 0:512])
    nc.sync.dma_start(out=qT_sb[:, 0:512], in_=qT[:, 0:512])
    nc.sync.dma_start(out=kT_sb[:, 512:1024], in_=kT[:, 512:1024])
    nc.sync.dma_start(out=v_sb[:, 0:8, :], in_=vt[:, 0:8, :])
    nc.sync.dma_start(out=kT_sb[:, 1024:2048], in_=kT[:, 1024:2048])
    nc.sync.dma_start(out=v_sb[:, 8:16, :], in_=vt[:, 8:16, :])
    nc.sync.dma_start(out=kT_sb[:, 2048:3072], in_=kT[:, 2048:3072])
    nc.sync.dma_start(out=v_sb[:, 16:24, :], in_=vt[:, 16:24, :])
    nc.sync.dma_start(out=kT_sb[:, 3072:S], in_=kT[:, 3072:S])
    nc.sync.dma_start(out=v_sb[:, 24:NKT, :], in_=vt[:, 24:NKT, :])
    nc.sync.dma_start(out=qT_sb[:, 512:HALF], in_=qT[:, 512:HALF])

    w1_sb = persist.tile([128, F], bf16, tag="w1")
    nc.sync.dma_start(out=w1_sb, in_=w1[:, :])
    w2_sb = persist.tile([128, FBLK, D], bf16, tag="w2")
    nc.sync.dma_start(out=w2_sb, in_=w2c[:, :, :])
    b1_sb = persist.tile([128, FBLK], f32, tag="b1")
    nc.sync.dma_start(out=b1_sb, in_=b1c[:, :])
    b2_t = persist.tile([128, 1], f32, tag="b2")
    nc.sync.dma_start(out=b2_t, in_=b2.ap().unsqueeze(1))
    g1_t = persist.tile([128, 1], f32, tag="g1")
    nc.sync.dma_start(out=g1_t, in_=g1.ap().unsqueeze(1))
    be1_t = persist.tile([128, 1], f32, tag="be1")
    nc.sync.dma_start(out=be1_t, in_=be1.ap().unsqueeze(1))
    g2_t = persist.tile([128, 1], f32, tag="g2")
    nc.sync.dma_start(out=g2_t, in_=g2.ap().unsqueeze(1))
    be2_t = persist.tile([128, 1], f32, tag="be2")
    nc.sync.dma_start(out=be2_t, in_=be2.ap().unsqueeze(1))

    # ---------------- constants ----------------
    # ones/D with M=128 so stat matmuls produce the mean replicated on every
    # partition -> no separate partition-broadcast step.
    ones_mm = persist.tile([128, 128], bf16, tag="ones_mm")
    nc.gpsimd.memset(ones_mm, 1.0 / D)

    # ---------------- post-attention phase as spreadable op list ----------------
    def make_post_ops(rows0, xz, c0, c1, act_rstd=False):
        """Closures for LN1 + FFN + residual + LN2 + store of columns [c0:c1) of
        the block whose attention output (and its square) sit in xz [128,2,*] bf16.

        act_rstd=True computes rstd via ACT Ln/Exp instead of the DVE quake
        sequence — used ONLY for the final block's chains, where ACT is idle
        (no exps left) and the DVE queue is the latency bottleneck."""
        N = c1 - c0
        cols = slice(c0, c1)
        st = {}
        ops = []

        def ln_ops(src0, src1, dst_key, g_t, be_t, key):
            """LayerNorm over partitions; src0/src1 are lambdas returning [128,N]
            APs (value and value^2); normalized result lands in st[dst_key].
            rstd comes from the bf16 quake rsqrt + one f32 Newton step on DVE."""

            def s1():
                st[key + "mu"] = misc_ps.tile([128, N], f32, tag="misc", name="ps_mu")
                nc.tensor.matmul(st[key + "mu"], ones_mm, src0())

            def s2():
                st[key + "ms"] = misc_ps.tile([128, N], f32, tag="misc", name="ps_ms")
                nc.tensor.matmul(st[key + "ms"], ones_mm, src1())

            def s3():
                t = st_pool.tile([128, 2, N], bf16, tag="st", name="st")
                st[key + "st"] = t
                nc.vector.tensor_copy(t[:, 0, :], st[key + "mu"])

            def s4():
                t = st[key + "st"]
                # var+eps = (ms + eps) - mu^2, into the bf16 stat tile
                nc.vector.tensor_tensor(t[:, 1, :], t[:, 0, :], t[:, 0, :], ALU.mult)
                nc.vector.scalar_tensor_tensor(
                    t[:, 1, :], st[key + "ms"], EPS, t[:, 1, :], ALU.add, ALU.subtract
                )

            def s5():
                # quake: y0 = bitcast(QMAGIC - (bits(var) >> 1)) in bf16-space
                t = st[key + "st"]
                vbits = t[:, 1, :].bitcast(u16)
                q0 = st_pool.tile([128, N], bf16, tag="q0", name="q0")
                q0b = q0.bitcast(u16)
                # QMAGIC - x == (x ^ 0xFFFF) - (0xFFFF - QMAGIC); intermediates
                # stay in [0, 0xFFFF] whether uint16 math wraps or saturates.
                # (bitwise and arith ALU ops can't be fused in one instruction)
                nc.vector.tensor_scalar(q0b, vbits, 1, None, ALU.logical_shift_right)
                nc.vector.tensor_scalar(q0b, q0b, 0xFFFF, None, ALU.bitwise_xor)
                nc.vector.tensor_scalar(q0b, q0b, 0xFFFF - QMAGIC, None, ALU.subtract)
                st[key + "q0"] = q0

            def s5b():
                # one Newton step in f32: rstd = y0*(1.5 - 0.5*var*y0^2)
                t = st[key + "st"]
                q0 = st[key + "q0"]
                tn = st_pool.tile([128, N], f32, tag="tn", name="tn")
                nc.vector.tensor_tensor(tn, t[:, 1, :], q0, ALU.mult)
                nc.vector.tensor_tensor(tn, tn, q0, ALU.mult)
                nc.vector.tensor_scalar(tn, tn, -0.5, 1.5, ALU.mult, ALU.add)
                nc.vector.tensor_tensor(tn, q0, tn, ALU.mult)
                st[key + "rstd"] = tn

            def s5_act():
                # rstd = exp(-0.5 * ln(var + eps)) on the otherwise-idle ACT
                t = st[key + "st"]
                nc.scalar.activation(t[:, 1, :], t[:, 1, :], AF.Ln)
                nc.scalar.activation(t[:, 1, :], t[:, 1, :], AF.Exp, scale=-0.5)
                st[key + "rstd"] = t[:, 1, :]

            def s6():
                t = st[key + "st"]
                dst = st[dst_key]
                nc.vector.tensor_tensor(dst, src0(), t[:, 0, :], ALU.subtract)
                nc.vector.tensor_tensor(dst, dst, st[key + "rstd"], ALU.mult)
                nc.vector.tensor_scalar(dst, dst, g_t, be_t, ALU.mult, ALU.add)

            if act_rstd:
                return [s1, s2, s3, s4, s5_act, s6]
            return [s1, s2, s3, s4, s5, s5b, s6]

        def a0():
            st["x1"] = x_pool.tile([128, N], bf16, tag="x", name="x1")

        ops.append(a0)
        ops.extend(
            ln_ops(lambda: xz[:, 0, cols], lambda: xz[:, 1, cols], "x1", g1_t, be1_t, "l1")
        )

        def f0():
            st["ffn"] = acc_ps.tile([128, N], f32, tag="acc", name="ps_ffn")

        ops.append(f0)
        for fb in range(FBLK):
            def fchunk(fb=fb):
                ps_h = misc_ps.tile([128, N], f32, tag="misc", name="ps_h")
                nc.tensor.matmul(ps_h, w1_sb[:, fb * 128 : (fb + 1) * 128], st["x1"])
                h = h_pool.tile([128, N], bf16, tag="h", name="h")
                if act_rstd:
                    # tail: ACT is idle, DVE is the latency bottleneck
                    nc.scalar.activation(h, ps_h, AF.Relu, bias=b1_sb[:, fb : fb + 1])
                else:
                    # relu(x + b1): fused add+max on DVE keeps ACT free for exp
                    nc.vector.tensor_scalar(h, ps_h, b1_sb[:, fb : fb + 1], 0.0, ALU.add, ALU.max)
                nc.tensor.matmul(
                    st["ffn"],
                    w2_sb[:, fb, :],
                    h,
                    start=(fb == 0),
                    stop=(fb == FBLK - 1),
                    skip_group_check=True,
                )

            ops.append(fchunk)

        def r0():
            z = xz_pool.tile([128, 2, N], bf16, tag="xz", name="z")
            st["z"] = z
            nc.vector.scalar_tensor_tensor(z[:, 0, :], st["ffn"], b2_t, st["x1"], ALU.add, ALU.add)
            if act_rstd:
                nc.scalar.activation(z[:, 1, :], z[:, 0, :], AF.Square)
            else:
                nc.vector.tensor_tensor(z[:, 1, :], z[:, 0, :], z[:, 0, :], ALU.mult)
            st["y2"] = y_pool.tile([128, N], bf16, tag="y", name="y2")

        ops.append(r0)
        ops.extend(
            ln_ops(lambda: st["z"][:, 0, :], lambda: st["z"][:, 1, :], "y2", g2_t, be2_t, "l2")
        )

        def out_dma():
            nc.sync.dma_start(out=outT[:, rows0 + c0 : rows0 + c1], in_=st["y2"])

        ops.append(out_dma)
        return ops

    # ---------------- software-pipelined main loop ----------------
    pending = deque()  # post ops of the previous block

    def pop(n=1):
        for _ in range(n):
            if pending:
                pending.popleft()()

    for bi, (r0, r1, ks) in enumerate(BLOCKS):
        N = r1 - r0
        nsup = NKT // ks
        ps_attn = acc_ps.tile([128, N], f32, tag="acc")
        prev_p = None
        prev_base = 0
        for sp in range(nsup):
            ps_s = score_ps.tile([128, ks, N], f32, tag="score")
            for hh in range(ks):
                jk = ks * sp + hh
                nc.tensor.matmul(
                    ps_s[:, hh, :], kT_sb[:, jk * 128 : (jk + 1) * 128], qT_sb[:, r0:r1]
                )
                if hh == 1:
                    pop()
            p_sb = p_pool.tile([128, ks, N], bf16, tag="p")
            nc.scalar.activation(p_sb, ps_s, AF.Exp, scale=INV_SQRT_D)
            # One-slot skew: accumulate the PREVIOUS slot's P@v so the PE never
            # waits on this slot's exp.
            if prev_p is not None:
                for hh in range(ks):
                    jk = prev_base + hh
                    nc.tensor.matmul(
                        ps_attn,
                        v_sb[:, jk, :],
                        prev_p[:, hh, :],
                        start=(jk == 0),
                        stop=False,
                        skip_group_check=True,
                    )
                    if hh == 1:
                        pop()
            prev_p = p_sb
            prev_base = ks * sp
            pop(2)
        for hh in range(ks):  # drain the skewed last slot
            jk = prev_base + hh
            nc.tensor.matmul(
                ps_attn,
                v_sb[:, jk, :],
                prev_p[:, hh, :],
                start=False,
                stop=(hh == ks - 1),
                skip_group_check=True,
            )
        # Eagerly spill the attention accumulator (and its square for the LN1
        # stats) so its psum bank frees for the next block. Remaining post ops
        # carry over into the next block's slots instead of clumping.
        xz = xz_pool.tile([128, 2, N], bf16, tag="xz", name="xz")
        if bi == len(BLOCKS) - 1:
            # final block: exps are done, spill via the idle ACT engine
            nc.scalar.activation(xz[:, 0, :], ps_attn, AF.Copy)
            nc.scalar.activation(xz[:, 1, :], ps_attn, AF.Square)
        else:
            nc.vector.tensor_copy(xz[:, 0, :], ps_attn)
            nc.vector.tensor_tensor(xz[:, 1, :], xz[:, 0, :], xz[:, 0, :], ALU.mult)
        if bi < len(BLOCKS) - 1:
            pending.extend(make_post_ops(r0, xz, 0, N))
        else:
            # split the final block's post phase into two half-width chains so the
            # kernel tail pipelines instead of one long dependency chain; rstd on
            # the now-idle ACT engine to unload the DVE queue
            opsA = make_post_ops(r0, xz, 0, N // 2, act_rstd=True)
            opsB = make_post_ops(r0, xz, N // 2, N, act_rstd=True)
            for a, b in zip(opsA, opsB):
                pending.append(a)
                pending.append(b)
    while pending:
        pending.popleft()()


def _patched_act_tables(module_arch):
    """Collapse the ACT table choice to the one set containing exp (+relu/copy
    fillers) so the kernel never swaps table sets (~2.7us per swap). Positions are
    preserved because act_func_set_id indexes the original act_info.json order."""
    from concourse.hw_specs import get_activation_tables

    tables = get_activation_tables(module_arch)
    keep = "natural_log_exp_and_others"
    if keep in tables:
        return {
            name: (funcs if name == keep else set())
            for name, funcs in tables.items()
        }
    return tables


def build():
    nc = bacc.Bacc("TRN2", target_bir_lowering=False, debug=False, num_devices=N_CORES)
    with tile.TileContext(nc) as tc:
        with ExitStack() as ctx:
            _emit(nc, tc, ctx)
    import concourse.bacc as bacc_mod

    orig = bacc_mod.get_activation_tables
    bacc_mod.get_activation_tables = _patched_act_tables
    try:
        nc.compile()
    finally:
        bacc_mod.get_activation_tables = orig
    return nc


_CACHE = {}


def _get_nc():
    if "nc" not in _CACHE:
        _CACHE["nc"] = build()
    return _CACHE["nc"]


def run(inputs, trace=False, trace_kwargs=None):
    """Run on 8 cores; returns (full_output, BassKernelResults)."""
    nc = _get_nc()
    bf = ml_dtypes.bfloat16
    q = np.asarray(inputs["q"], dtype=np.float32)
    k = np.asarray(inputs["k"], dtype=np.float32)
    v = np.asarray(inputs["v"], dtype=np.float32)
    w1_b = np.asarray(inputs["w1"], dtype=np.float32).astype(bf)
    w2c = (
        np.asarray(inputs["w2"], dtype=np.float32)
        .reshape(FBLK, 128, D)
        .transpose(1, 0, 2)
        .astype(bf)
    )
    b1c = np.ascontiguousarray(
        np.asarray(inputs["b1"], dtype=np.float32).reshape(FBLK, 128).T
    )
    scal = {
        n: np.ascontiguousarray(np.asarray(inputs[n], dtype=np.float32))
        for n in ("b2", "g1", "be1", "g2", "be2")
    }
    in_maps = []
    for c in range(N_CORES):
        b, h = divmod(c, 2)
        m = {"w1": w1_b, "w2c": w2c, "b1c": b1c, **scal}
        m["qT"] = np.ascontiguousarray(q[b, h * HALF : (h + 1) * HALF, :].T).astype(bf)
        m["kT"] = np.ascontiguousarray(k[b].T).astype(bf)
        m["vt"] = v[b].reshape(NKT, 128, D).transpose(1, 0, 2).astype(bf)
        in_maps.append(m)
    res = run_bass_kernel_spmd(
        nc, in_maps, list(range(N_CORES)), trace=trace, **(trace_kwargs or {})
    )
    full = np.empty((B, S, D), dtype=np.float32)
    for c in range(N_CORES):
        b, h = divmod(c, 2)
        full[b, h * HALF : (h + 1) * HALF, :] = (
            res.results[c]["outT"].astype(np.float32).T
        )
    return full, res


def kernel(**inputs):
    full, _ = run(inputs, trace=False)
    return full


# revision 27
# speedup vs baseline: 1.2109x; 1.2090x over previous
"""Trainium2 Bass kernel for a single-layer transformer block (attention + FFN + 2x LayerNorm).

Shapes (hardcoded): q,k,v [4,4096,128] fp32; w1 [128,512]; w2 [512,128]; out [4,4096,128].

Sharding: 8 cores; core c handles batch c//2, q-rows half c%2 (2048 rows each).
k/v for the batch are replicated on both cores of the pair. Pure data-parallel SPMD,
no collectives.

Host-side marshalling (inside kernel(), before the device kernel runs): q and k are
pre-transposed to [d, rows] layout and cast to bf16, v/w2 are pre-tiled into the
[128-partition, tile, 128] layout the PE consumes, so the device kernel does ZERO
on-chip transposes or dtype casts. The output is produced transposed ([d, rows] bf16)
and the host transposes/casts it back.

Per-core device algorithm (activations TRANSPOSED: [feature/kpos on partitions, rows free]):
  - slot over 2 kpos tiles: scores_T[kpos, rows] = kT_blk.T @ qT into a 2-bank psum
    tile (double-buffered: exp(n) overlaps scores(n+1); a single-buffered 4-bank tile
    was tried and serializes scores->exp->scores, costing ~1.2us/slot of PE idle),
    then ONE exp over the 1024 free elements (amortizes the ~550ns fixed ACT cost).
  - P = exp(scores / sqrt(d))    (max-subtraction unneeded: logits ~N(0,1); the
    softmax denominator cancels in LayerNorm scale-invariance)
  - attn_T[d, rows] += v_blk.T.T @ P_blk   (PE accumulation, bf16)
  - LN over d (=partitions): mean/meansq via ones-matmul with M=128 so the stats land
    REPLICATED across all partitions (no broadcast matmul); rstd = rsqrt(var+eps) via
    the bf16 quake bit-trick + one f32 Newton step, entirely on DVE (keeps ACT free
    for exp, which is the bottleneck engine).
  - FFN: h1T = w1.T @ xT (+b1, relu on DVE), ffnT = w2_blk.T @ h1T accumulated (PE).
  - residual + LN2, DMA the transposed bf16 result straight out.

q blocks are 512,512,512,256,256 columns: the small final block shortens the kernel
tail (its post-LN/FFN chain has no attention stream left to hide behind), and its
post phase is further split into two 128-column chains that pipeline against each
other.  Post-attention ops of block i spread across the attention slots of block i+1.
"""

import sys

sys.path.insert(0, "/opt/trn_rl_repo")

from collections import deque
from contextlib import ExitStack

import ml_dtypes
import numpy as np

import concourse.bass as bass  # noqa: F401
from concourse import bacc
import concourse.tile as tile
import concourse.mybir as mybir
from concourse.bass_utils import run_bass_kernel_spmd
from concourse.masks import make_identity

B, S, D, F = 4, 4096, 128, 512
N_CORES = 8
HALF = S // 2          # q rows per core
NKT = S // 128         # 32 kpos tiles
FBLK = F // 128        # 4 FFN chunks
EPS = 1e-5
INV_SQRT_D = float(1.0 / np.sqrt(D))
# (start, end, kpos-tiles-per-slot): narrow late blocks shorten the kernel tail;
# widening their slots (4 kpos tiles x 256 cols = same 1024-elem exp, same 2 psum
# banks) keeps the per-element ACT exp cost identical to the 512-col blocks.
BLOCKS = [
    (0, 512, 2),
    (512, 1024, 2),
    (1024, 1536, 2),
    (1536, 1792, 4),
    (1792, 2048, 4),
]

f32 = mybir.dt.float32
bf16 = mybir.dt.bfloat16
u16 = mybir.dt.uint16
AF = mybir.ActivationFunctionType
ALU = mybir.AluOpType

# quake rsqrt magic for bf16 (top 16 bits of the f32 magic 0x5f3759df)
QMAGIC = 0x5F37


def _emit(nc, tc, ctx):
    qT = nc.dram_tensor("qT", [D, HALF], bf16, kind="ExternalInput")
    kT = nc.dram_tensor("kT", [D, S], bf16, kind="ExternalInput")
    vt = nc.dram_tensor("vt", [128, NKT, D], bf16, kind="ExternalInput")
    w1 = nc.dram_tensor("w1", [D, F], bf16, kind="ExternalInput")
    w2c = nc.dram_tensor("w2c", [128, FBLK, D], bf16, kind="ExternalInput")
    b1c = nc.dram_tensor("b1c", [128, FBLK], f32, kind="ExternalInput")
    b2 = nc.dram_tensor("b2", [D], f32, kind="ExternalInput")
    g1 = nc.dram_tensor("g1", [D], f32, kind="ExternalInput")
    be1 = nc.dram_tensor("be1", [D], f32, kind="ExternalInput")
    g2 = nc.dram_tensor("g2", [D], f32, kind="ExternalInput")
    be2 = nc.dram_tensor("be2", [D], f32, kind="ExternalInput")
    outT = nc.dram_tensor("outT", [D, HALF], bf16, kind="ExternalOutput")

    # ---------------- pools ----------------
    persist = ctx.enter_context(tc.tile_pool(name="persist", bufs=1))
    p_pool = ctx.enter_context(tc.tile_pool(name="p", bufs=3))
    xz_pool = ctx.enter_context(tc.tile_pool(name="xz", bufs=4))
    x_pool = ctx.enter_context(tc.tile_pool(name="x", bufs=4))
    h_pool = ctx.enter_context(tc.tile_pool(name="h", bufs=4))
    st_pool = ctx.enter_context(tc.tile_pool(name="st", bufs=6))
    y_pool = ctx.enter_context(tc.tile_pool(name="y", bufs=3))

    # PSUM: score 2x2 banks (double-buffered so exp(n) overlaps scores(n+1))
    # + acc 2x1 + misc 2x1 = 8 banks exactly.
    score_ps = ctx.enter_context(tc.tile_pool(name="score_ps", bufs=2, space="PSUM"))
    acc_ps = ctx.enter_context(tc.tile_pool(name="acc_ps", bufs=2, space="PSUM"))
    misc_ps = ctx.enter_context(tc.tile_pool(name="misc_ps", bufs=2, space="PSUM"))

    # ---------------- big activations, startup-ordered DMAs ----------------
    kT_sb = persist.tile([128, S], bf16, tag="kT")
    qT_sb = persist.tile([128, HALF], bf16, tag="qT")
    v_sb = persist.tile([128, NKT, D], bf16, tag="v")

    # The first super-slot needs kT tiles 0-3 and qT block 0; feed the queue in
    # consumption order with fine chunks so the PE starts after ~200KB.
    nc.sync.dma_start(out=kT_sb[:, 0:512], in_=kT[:, 0:512])
    nc.sync.dma_start(out=qT_sb[:, 0:512], in_=qT[:, 0:512])
    nc.sync.dma_start(out=kT_sb[:, 512:1024], in_=kT[:, 512:1024])
    nc.sync.dma_start(out=v_sb[:, 0:8, :], in_=vt[:, 0:8, :])
    nc.sync.dma_start(out=kT_sb[:, 1024:2048], in_=kT[:, 1024:2048])
    nc.sync.dma_start(out=v_sb[:, 8:16, :], in_=vt[:, 8:16, :])
    nc.sync.dma_start(out=kT_sb[:, 2048:3072], in_=kT[:, 2048:3072])
    nc.sync.dma_start(out=v_sb[:, 16:24, :], in_=vt[:, 16:24, :])
    nc.sync.dma_start(out=kT_sb[:, 3072:S], in_=kT[:, 3072:S])
    nc.sync.dma_start(out=v_sb[:, 24:NKT, :], in_=vt[:, 24:NKT, :])
    nc.sync.dma_start(out=qT_sb[:, 512:HALF], in_=qT[:, 512:HALF])

    w1_sb = persist.tile([128, F], bf16, tag="w1")
    nc.sync.dma_start(out=w1_sb, in_=w1[:, :])
    w2_sb = persist.tile([128, FBLK, D], bf16, tag="w2")
    nc.sync.dma_start(out=w2_sb, in_=w2c[:, :, :])
    b1_sb = persist.tile([128, FBLK], f32, tag="b1")
    nc.sync.dma_start(out=b1_sb, in_=b1c[:, :])
    b2_t = persist.tile([128, 1], f32, tag="b2")
    nc.sync.dma_start(out=b2_t, in_=b2.ap().unsqueeze(1))
    g1_t = persist.tile([128, 1], f32, tag="g1")
    nc.sync.dma_start(out=g1_t, in_=g1.ap().unsqueeze(1))
    be1_t = persist.tile([128, 1], f32, tag="be1")
    nc.sync.dma_start(out=be1_t, in_=be1.ap().unsqueeze(1))
    g2_t = persist.tile([128, 1], f32, tag="g2")
    nc.sync.dma_start(out=g2_t, in_=g2.ap().unsqueeze(1))
    be2_t = persist.tile([128, 1], f32, tag="be2")
    nc.sync.dma_start(out=be2_t, in_=be2.ap().unsqueeze(1))

    # ---------------- constants ----------------
    # ones/D with M=128 so stat matmuls produce the mean replicated on every
    # partition -> no separate partition-broadcast step.
    ones_mm = persist.tile([128, 128], bf16, tag="ones_mm")
    nc.gpsimd.memset(ones_mm, 1.0 / D)
    # centering matmul (I - 11^T/D): xc = cen_mm.T @ x subtracts the partition
    # mean in a single PE pass (both values exact in bf16)
    ident = persist.tile([128, 128], f32, tag="ident")
    make_identity(nc, ident)
    cen_mm = persist.tile([128, 128], bf16, tag="cen_mm")
    nc.gpsimd.memset(cen_mm, -1.0 / D)
    nc.vector.tensor_tensor(cen_mm, cen_mm, ident, ALU.add)

    # ---------------- post-attention phase as spreadable op list ----------------
    def make_post_ops(rows0, xz, c0, c1, act_rstd=False):
        """Closures for LN1 + FFN + residual + LN2 + store of columns [c0:c1) of
        the block whose attention output sits in xz [128,*] bf16.

        act_rstd=True computes rstd via ACT Ln/Exp instead of the DVE quake
        sequence — used ONLY for the final block's chains, where ACT is idle
        (no exps left) and the DVE queue is the latency bottleneck."""
        N = c1 - c0
        cols = slice(c0, c1)
        st = {}
        ops = []

        def ln_ops(src, dst_key, g_t, be_t, key):
            """LayerNorm over partitions; src is a lambda returning the [128,N]
            input AP; normalized result lands in st[dst_key].  The mean-subtract
            is a single (I - 11^T/D) PE matmul; var = mean(xc^2); rstd from the
            bf16 quake rsqrt + one f32 Newton step on DVE (or ACT Ln/Exp)."""

            def c1():
                st[key + "c"] = misc_ps.tile([128, N], f32, tag="misc", name="ps_c")
                nc.tensor.matmul(st[key + "c"], cen_mm, src())

            def c2():
                t = st_pool.tile([128, 2, N], bf16, tag="st", name="xcq")
                st[key + "xc"] = t
                nc.vector.tensor_copy(t[:, 0, :], st[key + "c"])
                nc.vector.tensor_tensor(t[:, 1, :], t[:, 0, :], t[:, 0, :], ALU.mult)

            def c3():
                st[key + "v"] = misc_ps.tile([128, N], f32, tag="misc", name="ps_v")
                nc.tensor.matmul(st[key + "v"], ones_mm, st[key + "xc"][:, 1, :])

            def c4():
                # var -> SBUF bf16 (overwrites the spent xc^2 lane), then quake:
                # y0 = bitcast(QMAGIC - (bits(var) >> 1)) in bf16-space.
                # QMAGIC - x == (x ^ 0xFFFF) - (0xFFFF - QMAGIC); intermediates
                # stay in [0, 0xFFFF] whether uint16 math wraps or saturates.
                # (bitwise and arith ALU ops can't be fused in one instruction)
                t = st[key + "xc"]
                nc.vector.tensor_copy(t[:, 1, :], st[key + "v"])
                vbits = t[:, 1, :].bitcast(u16)
                q0 = st_pool.tile([128, N], bf16, tag="q0", name="q0")
                q0b = q0.bitcast(u16)
                nc.vector.tensor_scalar(q0b, vbits, 1, None, ALU.logical_shift_right)
                nc.vector.tensor_scalar(q0b, q0b, 0xFFFF, None, ALU.bitwise_xor)
                nc.vector.tensor_scalar(q0b, q0b, 0xFFFF - QMAGIC, None, ALU.subtract)
                st[key + "q0"] = q0

            def c5():
                # one Newton step in f32: rstd = y0*(1.5 - 0.5*var*y0^2)
                t = st[key + "xc"]
                q0 = st[key + "q0"]
                tn = st_pool.tile([128, N], f32, tag="tn", name="tn")
                nc.vector.tensor_tensor(tn, t[:, 1, :], q0, ALU.mult)
                nc.vector.tensor_tensor(tn, tn, q0, ALU.mult)
                nc.vector.tensor_scalar(tn, tn, -0.5, 1.5, ALU.mult, ALU.add)
                nc.vector.tensor_tensor(tn, q0, tn, ALU.mult)
                st[key + "rstd"] = tn

            def c4_act():
                # rstd = exp(-0.5 * ln(var)) on the otherwise-idle ACT, reading
                # the psum var directly (skips the SBUF copy hop)
                rs = st_pool.tile([128, N], bf16, tag="q0", name="rstd")
                nc.scalar.activation(rs, st[key + "v"], AF.Ln)
                nc.scalar.activation(rs, rs, AF.Exp, scale=-0.5)
                st[key + "rstd"] = rs

            def c6():
                t = st[key + "xc"]
                dst = st[dst_key]
                nc.vector.tensor_tensor(dst, t[:, 0, :], st[key + "rstd"], ALU.mult)
                nc.vector.tensor_scalar(dst, dst, g_t, be_t, ALU.mult, ALU.add)

            if act_rstd:
                return [c1, c2, c3, c4_act, c6]
            return [c1, c2, c3, c4, c5, c6]

        def a0():
            st["x1"] = x_pool.tile([128, N], bf16, tag="x", name="x1")

        ops.append(a0)
        ops.extend(ln_ops(lambda: xz[:, cols], "x1", g1_t, be1_t, "l1"))

        def f0():
            st["ffn"] = acc_ps.tile([128, N], f32, tag="acc", name="ps_ffn")

        ops.append(f0)
        for fb in range(FBLK):
            def fchunk(fb=fb):
                ps_h = misc_ps.tile([128, N], f32, tag="misc", name="ps_h")
                nc.tensor.matmul(ps_h, w1_sb[:, fb * 128 : (fb + 1) * 128], st["x1"])
                h = h_pool.tile([128, N], bf16, tag="h", name="h")
                if act_rstd:
                    # tail: ACT is idle, DVE is the latency bottleneck
                    nc.scalar.activation(h, ps_h, AF.Relu, bias=b1_sb[:, fb : fb + 1])
                else:
                    # relu(x + b1): fused add+max on DVE keeps ACT free for exp
                    nc.vector.tensor_scalar(h, ps_h, b1_sb[:, fb : fb + 1], 0.0, ALU.add, ALU.max)
                nc.tensor.matmul(
                    st["ffn"],
                    w2_sb[:, fb, :],
                    h,
                    start=(fb == 0),
                    stop=(fb == FBLK - 1),
                    skip_group_check=True,
                )

            ops.append(fchunk)

        def r0():
            z = xz_pool.tile([128, N], bf16, tag="xz", name="z")
            st["z"] = z
            nc.vector.scalar_tensor_tensor(z, st["ffn"], b2_t, st["x1"], ALU.add, ALU.add)
            st["y2"] = y_pool.tile([128, N], bf16, tag="y", name="y2")

        ops.append(r0)
        ops.extend(ln_ops(lambda: st["z"], "y2", g2_t, be2_t, "l2"))

        def out_dma():
            nc.sync.dma_start(out=outT[:, rows0 + c0 : rows0 + c1], in_=st["y2"])

        ops.append(out_dma)
        return ops

    # ---------------- software-pipelined main loop ----------------
    pending = deque()  # post ops of the previous block

    def pop(n=1):
        for _ in range(n):
            if pending:
                pending.popleft()()

    for bi, (r0, r1, ks) in enumerate(BLOCKS):
        N = r1 - r0
        nsup = NKT // ks
        ps_attn = acc_ps.tile([128, N], f32, tag="acc")
        prev_p = None
        prev_base = 0
        for sp in range(nsup):
            ps_s = score_ps.tile([128, ks, N], f32, tag="score")
            for hh in range(ks):
                jk = ks * sp + hh
                nc.tensor.matmul(
                    ps_s[:, hh, :], kT_sb[:, jk * 128 : (jk + 1) * 128], qT_sb[:, r0:r1]
                )
                if hh == 1:
                    pop()
            p_sb = p_pool.tile([128, ks, N], bf16, tag="p")
            nc.scalar.activation(p_sb, ps_s, AF.Exp, scale=INV_SQRT_D)
            # One-slot skew: accumulate the PREVIOUS slot's P@v so the PE never
            # waits on this slot's exp.
            if prev_p is not None:
                for hh in range(ks):
                    jk = prev_base + hh
                    nc.tensor.matmul(
                        ps_attn,
                        v_sb[:, jk, :],
                        prev_p[:, hh, :],
                        start=(jk == 0),
                        stop=False,
                        skip_group_check=True,
                    )
                    if hh == 1:
                        pop()
            prev_p = p_sb
            prev_base = ks * sp
            pop(2)
        for hh in range(ks):  # drain the skewed last slot
            jk = prev_base + hh
            nc.tensor.matmul(
                ps_attn,
                v_sb[:, jk, :],
                prev_p[:, hh, :],
                start=False,
                stop=(hh == ks - 1),
                skip_group_check=True,
            )
        # Eagerly spill the attention accumulator (and its square for the LN1
        # stats) so its psum bank frees for the next block. Remaining post ops
        # carry over into the next block's slots instead of clumping.
        xz = xz_pool.tile([128, N], bf16, tag="xz", name="xz")
        if bi == len(BLOCKS) - 1:
            # final block: exps are done, spill via the idle ACT engine
            nc.scalar.activation(xz, ps_attn, AF.Copy)
        else:
            nc.vector.tensor_copy(xz, ps_attn)
        if bi < len(BLOCKS) - 1:
            pending.extend(make_post_ops(r0, xz, 0, N))
        else:
            # split the final block's post phase into two half-width chains so the
            # kernel tail pipelines instead of one long dependency chain; rstd on
            # the now-idle ACT engine to unload the DVE queue
            opsA = make_post_ops(r0, xz, 0, N // 2, act_rstd=True)
            opsB = make_post_ops(r0, xz, N // 2, N, act_rstd=True)
            for a, b in zip(opsA, opsB):
                pending.append(a)
                pending.append(b)
    while pending:
        pending.popleft()()


def _patched_act_tables(module_arch):
    """Collapse the ACT table choice to the one set containing exp (+relu/copy
    fillers) so the kernel never swaps table sets (~2.7us per swap). Positions are
    preserved because act_func_set_id indexes the original act_info.json order."""
    from concourse.hw_specs import get_activation_tables

    tables = get_activation_tables(module_arch)
    keep = "natural_log_exp_and_others"
    if keep in tables:
        return {
            name: (funcs if name == keep else set())
            for name, funcs in tables.items()
        }
    return tables


def build():
    nc = bacc.Bacc("TRN2", target_bir_lowering=False, debug=False, num_devices=N_CORES)
    with tile.TileContext(nc) as tc:
        with ExitStack() as ctx:
            _emit(nc, tc, ctx)
    import concourse.bacc as bacc_mod

    orig = bacc_mod.get_activation_tables
    bacc_mod.get_activation_tables = _patched_act_tables
    try:
        nc.compile()
    finally:
        bacc_mod.get_activation_tables = orig
    return nc


_CACHE = {}


def _get_nc():
    if "nc" not in _CACHE:
        _CACHE["nc"] = build()
    return _CACHE["nc"]


def run(inputs, trace=False, trace_kwargs=None):
    """Run on 8 cores; returns (full_output, BassKernelResults)."""
    nc = _get_nc()
    bf = ml_dtypes.bfloat16
    q = np.asarray(inputs["q"], dtype=np.float32)
    k = np.asarray(inputs["k"], dtype=np.float32)
    v = np.asarray(inputs["v"], dtype=np.float32)
    w1_b = np.asarray(inputs["w1"], dtype=np.float32).astype(bf)
    w2c = (
        np.asarray(inputs["w2"], dtype=np.float32)
        .reshape(FBLK, 128, D)
        .transpose(1, 0, 2)
        .astype(bf)
    )
    b1c = np.ascontiguousarray(
        np.asarray(inputs["b1"], dtype=np.float32).reshape(FBLK, 128).T
    )
    scal = {
        n: np.ascontiguousarray(np.asarray(inputs[n], dtype=np.float32))
        for n in ("b2", "g1", "be1", "g2", "be2")
    }
    in_maps = []
    for c in range(N_CORES):
        b, h = divmod(c, 2)
        m = {"w1": w1_b, "w2c": w2c, "b1c": b1c, **scal}
        m["qT"] = np.ascontiguousarray(q[b, h * HALF : (h + 1) * HALF, :].T).astype(bf)
        m["kT"] = np.ascontiguousarray(k[b].T).astype(bf)
        m["vt"] = v[b].reshape(NKT, 128, D).transpose(1, 0, 2).astype(bf)
        in_maps.append(m)
    res = run_bass_kernel_spmd(
        nc, in_maps, list(range(N_CORES)), trace=trace, **(trace_kwargs or {})
    )
    full = np.empty((B, S, D), dtype=np.float32)
    for c in range(N_CORES):
        b, h = divmod(c, 2)
        full[b, h * HALF : (h + 1) * HALF, :] = (
            res.results[c]["outT"].astype(np.float32).T
        )
    return full, res


def kernel(**inputs):
    full, _ = run(inputs, trace=False)
    return full
